# revision 9
# baseline (speedup 1.0000x reference)
"""CTRNN forward kernel for Trainium2 (8 NeuronCores, data-parallel over batch).

Reference computation (per step t, dt=0.02):
    h = h*(1-dt) + dt*(tanh(h) @ J.T + v_t @ Bmat.T)
    out_t = tanh(h) @ W_ro.T

Design (v3):
  - Per core: B_LOC=16 batch rows, hT layout (hidden on partitions, 4 row
    blocks of 128; batch on free dim). h lives in PSUM scaled by HSCALE=64
    (fp16 subnormal guard): H = 64h, y = tanh(H/64) via ACT's input scale.
  - h is double-buffered across two PSUM bank SETS (X at even steps, Y at
    odd steps), each set = 2 banks of [128, 32] (= 2 row blocks x 16 batch).
    Per step, per half g: one DVE scalar_tensor_tensor
        S_t[g] = 0.98*S_{t-1}[g] + bv_t[g]
    reads the PREVIOUS set, so it depends only on step t-1's matmuls --
    not on ACT -- and ACT(t-1) can read S_{t-1} concurrently.
  - 16 fp16 matmuls/step (J tiles [128,128] stationary, y [128,16] moving)
    accumulate into S_t, then 2 ACTs produce y_t = tanh(S_t/64) into a
    32-step fp16 ring. The MM issue order (PI below) was chosen by
    simulating the steady-state pipeline (MM drain 174ns + sem ~156ns +
    ACT ~273ns + sem ~45ns on the y loop); add_dep_helper(sync=False)
    chains pin the per-engine order against scheduler reordering.
  - bv outer products: vel is broadcast-DMA'd to all 128 partitions once
    per 128-step block, then 16 small DVE tensor_scalar multiplies
    (scalar = per-partition 64*dt*B column) build bvp -- no PE, no PSUM.
  - Readout batched per 32 steps: 4 accumulating MMs (lhsT = W_ro chunk
    [128,1], rhs = y ring [128,512]) -> PSUM [1,512] -> SBUF -> DRAM.
"""

import math
import sys

import numpy as np

sys.path.insert(0, "/opt/trn_rl_repo")

DT = 0.02
DECAY = 1.0 - DT          # 0.98
HSCALE = 64.0             # h kept as 64*h in PSUM (fp16 subnormal guard)
HIDDEN = 512
BATCH = 128
T_FULL = 1024
N_CORES = 8
B_LOC = BATCH // N_CORES  # 16
CB = HIDDEN // 128        # 4 row blocks / y chunks

# MM issue order (block b, chunk c); groups: bank A = blocks {0,1},
# bank B = blocks {2,3}. Found by steady-state pipeline search.
PI = [
    (1, 1), (0, 1), (1, 0), (2, 0), (2, 1), (0, 0), (0, 3), (1, 2),
    (0, 2), (1, 3), (3, 0), (3, 2), (3, 1), (3, 3), (2, 3), (2, 2),
]


def build_nc(T=T_FULL, lbv=128, ro=32):
    import concourse.bass as bass
    import concourse.tile as tile
    from concourse import bacc, mybir

    f32 = mybir.dt.float32
    f16 = mybir.dt.float16
    nc = bacc.Bacc()

    jt_h = nc.declare_dram_parameter("JT", [HIDDEN, HIDDEN], f16, isOutput=False)
    bmt_h = nc.declare_dram_parameter("bmt", [128, CB], f32, isOutput=False)
    wrt_h = nc.declare_dram_parameter("wrt", [128, CB], f16, isOutput=False)
    velt_h = nc.declare_dram_parameter("velT", [T, B_LOC], f16, isOutput=False)
    out_h = nc.declare_dram_parameter("out", [1, T * B_LOC], f32, isOutput=True)

    nblk = (T + lbv - 1) // lbv
    nro = (T + ro - 1) // ro
    rosz = ro * B_LOC  # 512 = one PSUM bank of fp32

    last = {}

    def chain(key, inst):
        last[key] = inst
        return inst

    with tile.TileContext(nc) as tc:
        with (
            tc.tile_pool(name="singles", bufs=1) as singles,
            tc.tile_pool(name="ybp", bufs=2) as ybp,
            tc.tile_pool(name="velp", bufs=2) as velp,
            tc.tile_pool(name="bvpp", bufs=2) as bvpp,
            tc.tile_pool(name="osbp", bufs=2) as osbp,
            tc.tile_pool(name="psum", bufs=1, space="PSUM") as pp,
        ):
            # ---- weights staging ----
            jt = singles.tile([128, CB, HIDDEN], f16, tag="jt")  # (64*dt*J)^T
            nc.sync.dma_start(out=jt, in_=jt_h.rearrange("(c p) i -> p c i", p=128))
            bmt = singles.tile([128, CB], f32, tag="bmt")  # 64*dt*Bmat columns
            nc.sync.dma_start(out=bmt, in_=bmt_h[:, :])
            wrt = singles.tile([128, CB], f16, tag="wrt")
            nc.sync.dma_start(out=wrt, in_=wrt_h[:, :])

            zrow = singles.tile([1, 512], f32, tag="zrow")
            chain("dve", nc.vector.memset(zrow, 0.0))

            y0 = singles.tile([128, CB, B_LOC], f16, tag="y0")
            chain(
                "dve",
                nc.vector.memset(y0.rearrange("p c b -> p (c b)").bitcast(f32), 0.0),
            )

            # h banks: z[g] = [128, 32] (blocks 2g, 2g+1), in-place
            z = [
                pp.tile([128, 2 * B_LOC], f32, tag=f"z{g}", name=f"psum_z{g}")
                for g in range(2)
            ]
            pjunk = pp.tile([1, 8], f32, tag="junk", name="psum_junk")

            def absorb(src):
                if src.dtype != f32:
                    src = src.bitcast(f32)
                chain(
                    "pe",
                    nc.tensor.matmul(
                        out=pjunk[0:1, 0:1],
                        lhsT=src,
                        rhs=src,
                        start=True,
                        stop=True,
                        skip_group_check=True,
                    ),
                )

            # claim + zero the h banks
            for g in range(2):
                chain(
                    "pe",
                    nc.tensor.matmul(
                        out=z[g],
                        lhsT=zrow[0:1, 0:128],
                        rhs=zrow[0:1, 0 : 2 * B_LOC],
                        start=True,
                        stop=True,
                        skip_group_check=True,
                    ),
                )

            absorb(jt[0:1, 0, 0:2])
            absorb(wrt[0:1, 0:2])
            absorb(bmt[0:1, 0:1])

            def dma_velb(t0):
                # vel for one lbv block, broadcast to all 128 partitions
                vb = velp.tile([128, lbv * B_LOC], f16, tag="velB")
                nc.sync.dma_start(
                    out=vb,
                    in_=velt_h[t0 : t0 + lbv, :]
                    .rearrange("t b -> (t b)")
                    .unsqueeze(0)
                    .partition_broadcast(128),
                )
                return vb

            def build_bv(r, vb, bvp_t):
                # pair r: chunk c = r // 4, quarter q = r % 4 (32 steps)
                c, q = divmod(r, 4)
                chain(
                    "dve",
                    nc.vector.tensor_scalar_mul(
                        out=bvp_t[:, q * 32 : (q + 1) * 32, c, :],
                        in0=vb[:, q * 512 : (q + 1) * 512].rearrange(
                            "p (t b) -> p t b", b=B_LOC
                        ),
                        scalar1=bmt[:, c : c + 1],
                    ),
                )

            def emit_readout(k, ytile):
                pro = pp.tile([1, rosz], f32, tag="ro", bufs=1, name="psum_ro")
                for c in range(CB):
                    chain(
                        "pe",
                        nc.tensor.matmul(
                            out=pro,
                            lhsT=wrt[:, c : c + 1],
                            rhs=ytile[:, c, :, :].rearrange("p t b -> p (t b)"),
                            start=(c == 0),
                            stop=(c == CB - 1),
                            skip_group_check=True,
                        ),
                    )
                osb = osbp.tile([1, rosz], f32, tag="osb", name="out_sb")
                chain("dve", nc.vector.tensor_copy(osb, pro))
                nc.sync.dma_start(
                    out=out_h[0:1, k * rosz : (k + 1) * rosz], in_=osb
                )

            # prologue: vel block 0 + its bv products
            velb = dma_velb(0)
            bvp_cur = bvpp.tile([128, lbv, CB, B_LOC], f16, tag="bvp")
            for r in range(16):
                build_bv(r, velb, bvp_cur)
            bvp_next = None
            velb_next = None

            yb_cur = None
            yb_prev = None
            for t in range(T):
                blk, j = divmod(t, lbv)
                rob, rj = divmod(t, ro)

                if rj == 0:
                    yb_prev = yb_cur
                    yb_cur = ybp.tile([128, CB, ro, B_LOC], f16, tag="yb")

                if j == 0 and blk + 1 < nblk:
                    velb_next = dma_velb(t + lbv)
                    bvp_next = bvpp.tile([128, lbv, CB, B_LOC], f16, tag="bvp")

                # spread next block's bv build: one op per 8 steps
                if blk + 1 < nblk and j >= 8 and j % 8 == 0:
                    build_bv(j // 8 - 1, velb_next, bvp_next)
                    if j == 120:
                        build_bv(15, velb_next, bvp_next)

                # batched readout of the previous 32-step block
                if rj == 4 and rob >= 1:
                    emit_readout(rob - 1, yb_prev)

                # ---- the step ----
                S = z
                if t == 0:
                    ysl = lambda c: y0[:, c, :]
                elif rj == 0:
                    ysl = lambda c: yb_prev[:, c, ro - 1, :]
                else:
                    ysl = lambda c, _s=rj - 1: yb_cur[:, c, _s, :]
                for g in range(2):
                    for b in (2 * g, 2 * g + 1):
                        for c in range(CB):
                            chain(
                                "pe",
                                nc.tensor.matmul(
                                    out=S[g][:, 16 * (b % 2) : 16 * (b % 2) + 16],
                                    lhsT=jt[:, c, 128 * b : 128 * (b + 1)],
                                    rhs=ysl(c),
                                    start=False,
                                    stop=False,
                                    skip_group_check=True,
                                ),
                            )
                    chain(
                        "dve",
                        nc.vector.scalar_tensor_tensor(
                            out=S[g],
                            in0=S[g],
                            scalar=float(DECAY),
                            in1=bvp_cur[:, j, 2 * g : 2 * g + 2, :].rearrange(
                                "p c b -> p (c b)"
                            ),
                            op0=mybir.AluOpType.mult,
                            op1=mybir.AluOpType.add,
                        ),
                    )
                    chain(
                        "sce",
                        nc.scalar.activation(
                            out=yb_cur[:, 2 * g : 2 * g + 2, rj, :],
                            in_=S[g].rearrange("p (c b) -> p c b", b=B_LOC),
                            func=mybir.ActivationFunctionType.Tanh,
                            scale=1.0 / HSCALE,
                        ),
                    )

                if j == lbv - 1 and blk + 1 < nblk:
                    bvp_cur = bvp_next
                    velb = velb_next

            emit_readout(nro - 1, yb_cur)

    nc.compile()
    return nc


_NC_CACHE = {}


def _get_nc(**kw):
    key = tuple(sorted(kw.items()))
    if key not in _NC_CACHE:
        _NC_CACHE[key] = build_nc(**kw)
    return _NC_CACHE[key]


def make_in_maps(vel, J, Bmat, W_ro):
    vel = np.asarray(vel, dtype=np.float32)[:, :, 0]          # [B, T]
    J = np.asarray(J, dtype=np.float32)
    Bmat = np.asarray(Bmat, dtype=np.float32)
    W_ro = np.asarray(W_ro, dtype=np.float32)

    jt = np.ascontiguousarray((HSCALE * DT / DECAY * J).T).astype(np.float16)
    bmt = np.ascontiguousarray(
        (HSCALE * DT * Bmat[:, 0]).reshape(CB, 128).T
    ).astype(np.float32)
    wrt = np.ascontiguousarray(W_ro[0].reshape(CB, 128).T).astype(np.float16)
    return [
        {
            "JT": jt,
            "bmt": bmt,
            "wrt": wrt,
            "velT": np.ascontiguousarray(
                vel[c * B_LOC : (c + 1) * B_LOC].T
            ).astype(np.float16),
        }
        for c in range(N_CORES)
    ]


def kernel(vel, J, Bmat, W_ro, _trace=False, **build_kw):
    from concourse.bass_utils import run_bass_kernel_spmd

    nc = _get_nc(**build_kw)
    in_maps = make_in_maps(vel, J, Bmat, W_ro)
    res = run_bass_kernel_spmd(nc, in_maps, list(range(N_CORES)), trace=_trace)
    # out[0, t*B_LOC + b] = readout(batch row b, step t)
    out = np.stack(
        [r["out"].reshape(T_FULL, B_LOC).T for r in res.results], axis=0
    ).reshape(BATCH, T_FULL)
    out = out[:, :, None].astype(np.float32)
    if _trace:
        kernel.last_results = res
    return out


kernel.last_results = None


# revision 10
# speedup vs baseline: 1.0255x; 1.0255x over previous
"""CTRNN forward kernel for Trainium2 (8 NeuronCores, data-parallel over batch).

Reference computation (per step t, dt=0.02):
    h = h*(1-dt) + dt*(tanh(h) @ J.T + v_t @ Bmat.T)
    out_t = tanh(h) @ W_ro.T

Design (v3):
  - Per core: B_LOC=16 batch rows, hT layout (hidden on partitions, 4 row
    blocks of 128; batch on free dim). h lives in PSUM scaled by HSCALE=64
    (fp16 subnormal guard): H = 64h, y = tanh(H/64) via ACT's input scale.
  - h is double-buffered across two PSUM bank SETS (X at even steps, Y at
    odd steps), each set = 2 banks of [128, 32] (= 2 row blocks x 16 batch).
    Per step, per half g: one DVE scalar_tensor_tensor
        S_t[g] = 0.98*S_{t-1}[g] + bv_t[g]
    reads the PREVIOUS set, so it depends only on step t-1's matmuls --
    not on ACT -- and ACT(t-1) can read S_{t-1} concurrently.
  - 16 fp16 matmuls/step (J tiles [128,128] stationary, y [128,16] moving)
    accumulate into S_t, then 2 ACTs produce y_t = tanh(S_t/64) into a
    32-step fp16 ring. The MM issue order (PI below) was chosen by
    simulating the steady-state pipeline (MM drain 174ns + sem ~156ns +
    ACT ~273ns + sem ~45ns on the y loop); add_dep_helper(sync=False)
    chains pin the per-engine order against scheduler reordering.
  - bv outer products: vel is broadcast-DMA'd to all 128 partitions once
    per 128-step block, then 16 small DVE tensor_scalar multiplies
    (scalar = per-partition 64*dt*B column) build bvp -- no PE, no PSUM.
  - Readout batched per 32 steps: 4 accumulating MMs (lhsT = W_ro chunk
    [128,1], rhs = y ring [128,512]) -> PSUM [1,512] -> SBUF -> DRAM.
"""

import math
import sys

import numpy as np

sys.path.insert(0, "/opt/trn_rl_repo")

DT = 0.02
DECAY = 1.0 - DT          # 0.98
HSCALE = 64.0             # h kept as 64*h in PSUM (fp16 subnormal guard)
HIDDEN = 512
BATCH = 128
T_FULL = 1024
N_CORES = 8
B_LOC = BATCH // N_CORES  # 16
CB = HIDDEN // 128        # 4 row blocks / y chunks

# MM issue order (block b, chunk c); groups: bank A = blocks {0,1},
# bank B = blocks {2,3}. Found by steady-state pipeline search.
PI = [
    (3, 2), (3, 3), (2, 3), (2, 2), (1, 2), (2, 0), (2, 1), (3, 1),
    (3, 0), (1, 1), (1, 3), (1, 0), (0, 0), (0, 2), (0, 3), (0, 1),
]


def build_nc(T=T_FULL, lbv=128, ro=32):
    import concourse.bass as bass
    import concourse.tile as tile
    from concourse import bacc, mybir

    f32 = mybir.dt.float32
    f16 = mybir.dt.float16
    nc = bacc.Bacc()

    jt_h = nc.declare_dram_parameter("JT", [HIDDEN, HIDDEN], f16, isOutput=False)
    bmt_h = nc.declare_dram_parameter("bmt", [128, CB], f32, isOutput=False)
    wrt_h = nc.declare_dram_parameter("wrt", [128, CB], f16, isOutput=False)
    velt_h = nc.declare_dram_parameter("velT", [T, B_LOC], f16, isOutput=False)
    out_h = nc.declare_dram_parameter("out", [1, T * B_LOC], f32, isOutput=True)

    nblk = (T + lbv - 1) // lbv
    nro = (T + ro - 1) // ro
    rosz = ro * B_LOC  # 512 = one PSUM bank of fp32

    last = {}

    def chain(key, inst):
        last[key] = inst
        return inst

    with tile.TileContext(nc) as tc:
        with (
            tc.tile_pool(name="singles", bufs=1) as singles,
            tc.tile_pool(name="ybp", bufs=2) as ybp,
            tc.tile_pool(name="velp", bufs=2) as velp,
            tc.tile_pool(name="bvpp", bufs=2) as bvpp,
            tc.tile_pool(name="osbp", bufs=2) as osbp,
            tc.tile_pool(name="psum", bufs=1, space="PSUM") as pp,
        ):
            # ---- weights staging ----
            jt = singles.tile([128, CB, HIDDEN], f16, tag="jt")  # (64*dt*J)^T
            nc.sync.dma_start(out=jt, in_=jt_h.rearrange("(c p) i -> p c i", p=128))
            bmt = singles.tile([128, CB], f32, tag="bmt")  # 64*dt*Bmat columns
            nc.sync.dma_start(out=bmt, in_=bmt_h[:, :])
            wrt = singles.tile([128, CB], f16, tag="wrt")
            nc.sync.dma_start(out=wrt, in_=wrt_h[:, :])

            zrow = singles.tile([1, 512], f32, tag="zrow")
            chain("dve", nc.vector.memset(zrow, 0.0))

            y0 = singles.tile([128, CB, B_LOC], f16, tag="y0")
            chain(
                "dve",
                nc.vector.memset(y0.rearrange("p c b -> p (c b)").bitcast(f32), 0.0),
            )

            # h bank sets: xy[s][g] = [128, 32] (blocks 2g, 2g+1)
            xy = [
                [
                    pp.tile([128, 2 * B_LOC], f32, tag=f"z{s}{g}", name=f"psum_z{s}{g}")
                    for g in range(2)
                ]
                for s in range(2)
            ]
            pjunk = pp.tile([1, 8], f32, tag="junk", name="psum_junk")

            def absorb(src):
                if src.dtype != f32:
                    src = src.bitcast(f32)
                chain(
                    "pe",
                    nc.tensor.matmul(
                        out=pjunk[0:1, 0:1],
                        lhsT=src,
                        rhs=src,
                        start=True,
                        stop=True,
                        skip_group_check=True,
                    ),
                )

            # claim + zero all four h banks
            for s in range(2):
                for g in range(2):
                    chain(
                        "pe",
                        nc.tensor.matmul(
                            out=xy[s][g],
                            lhsT=zrow[0:1, 0:128],
                            rhs=zrow[0:1, 0 : 2 * B_LOC],
                            start=True,
                            stop=True,
                            skip_group_check=True,
                        ),
                    )

            absorb(jt[0:1, 0, 0:2])
            absorb(wrt[0:1, 0:2])
            absorb(bmt[0:1, 0:1])

            def dma_velb(t0):
                # vel for one lbv block, broadcast to all 128 partitions
                vb = velp.tile([128, lbv * B_LOC], f16, tag="velB")
                nc.sync.dma_start(
                    out=vb,
                    in_=velt_h[t0 : t0 + lbv, :]
                    .rearrange("t b -> (t b)")
                    .unsqueeze(0)
                    .partition_broadcast(128),
                )
                return vb

            def build_bv(r, vb, bvp_t):
                # pair r: chunk c = r // 4, quarter q = r % 4 (32 steps)
                c, q = divmod(r, 4)
                chain(
                    "dve",
                    nc.vector.tensor_scalar_mul(
                        out=bvp_t[:, q * 32 : (q + 1) * 32, c, :],
                        in0=vb[:, q * 512 : (q + 1) * 512].rearrange(
                            "p (t b) -> p t b", b=B_LOC
                        ),
                        scalar1=bmt[:, c : c + 1],
                    ),
                )

            def emit_readout(k, ytile):
                pro = pp.tile([1, rosz], f32, tag="ro", bufs=1, name="psum_ro")
                for c in range(CB):
                    chain(
                        "pe",
                        nc.tensor.matmul(
                            out=pro,
                            lhsT=wrt[:, c : c + 1],
                            rhs=ytile[:, c, :, :].rearrange("p t b -> p (t b)"),
                            start=(c == 0),
                            stop=(c == CB - 1),
                            skip_group_check=True,
                        ),
                    )
                osb = osbp.tile([1, rosz], f32, tag="osb", name="out_sb")
                chain("dve", nc.vector.tensor_copy(osb, pro))
                nc.sync.dma_start(
                    out=out_h[0:1, k * rosz : (k + 1) * rosz], in_=osb
                )

            # prologue: vel block 0 + its bv products
            velb = dma_velb(0)
            bvp_cur = bvpp.tile([128, lbv, CB, B_LOC], f16, tag="bvp")
            for r in range(16):
                build_bv(r, velb, bvp_cur)
            bvp_next = None
            velb_next = None

            yb_cur = None
            yb_prev = None
            for t in range(T):
                blk, j = divmod(t, lbv)
                rob, rj = divmod(t, ro)

                if rj == 0:
                    yb_prev = yb_cur
                    yb_cur = ybp.tile([128, CB, ro, B_LOC], f16, tag="yb")

                if j == 0 and blk + 1 < nblk:
                    velb_next = dma_velb(t + lbv)
                    bvp_next = bvpp.tile([128, lbv, CB, B_LOC], f16, tag="bvp")

                # spread next block's bv build: one op per 8 steps
                if blk + 1 < nblk and j >= 8 and j % 8 == 0:
                    build_bv(j // 8 - 1, velb_next, bvp_next)
                    if j == 120:
                        build_bv(15, velb_next, bvp_next)

                # batched readout of the previous 32-step block
                if rj == 4 and rob >= 1:
                    emit_readout(rob - 1, yb_prev)

                # ---- the step ----
                S = xy[t % 2]
                Pv = xy[1 - t % 2]
                # decay+input per quarter-column so each range's last writer
                # is a single full-cover matmul (keeps ACT waits single)
                for b in (2, 3, 1, 0):
                    cs = slice(16 * (b % 2), 16 * (b % 2) + 16)
                    chain(
                        "dve",
                        nc.vector.scalar_tensor_tensor(
                            out=S[b // 2][:, cs],
                            in0=Pv[b // 2][:, cs],
                            scalar=float(DECAY),
                            in1=bvp_cur[:, j, b, :],
                            op0=mybir.AluOpType.mult,
                            op1=mybir.AluOpType.add,
                        ),
                    )
                if t == 0:
                    ysl = lambda c: y0[:, c, :]
                elif rj == 0:
                    ysl = lambda c: yb_prev[:, c, ro - 1, :]
                else:
                    ysl = lambda c, _s=rj - 1: yb_cur[:, c, _s, :]
                for b, c in PI:
                    chain(
                        "pe",
                        nc.tensor.matmul(
                            out=S[b // 2][:, 16 * (b % 2) : 16 * (b % 2) + 16],
                            lhsT=jt[:, c, 128 * b : 128 * (b + 1)],
                            rhs=ysl(c),
                            start=False,
                            stop=False,
                            skip_group_check=True,
                        ),
                    )
                for g in (1, 0):
                    chain(
                        "sce",
                        nc.scalar.activation(
                            out=yb_cur[:, 2 * g : 2 * g + 2, rj, :],
                            in_=S[g].rearrange("p (c b) -> p c b", b=B_LOC),
                            func=mybir.ActivationFunctionType.Tanh,
                            scale=1.0 / HSCALE,
                        ),
                    )

                if j == lbv - 1 and blk + 1 < nblk:
                    bvp_cur = bvp_next
                    velb = velb_next

            emit_readout(nro - 1, yb_cur)

    nc.compile()
    return nc


_NC_CACHE = {}


def _get_nc(**kw):
    key = tuple(sorted(kw.items()))
    if key not in _NC_CACHE:
        _NC_CACHE[key] = build_nc(**kw)
    return _NC_CACHE[key]


def make_in_maps(vel, J, Bmat, W_ro):
    vel = np.asarray(vel, dtype=np.float32)[:, :, 0]          # [B, T]
    J = np.asarray(J, dtype=np.float32)
    Bmat = np.asarray(Bmat, dtype=np.float32)
    W_ro = np.asarray(W_ro, dtype=np.float32)

    jt = np.ascontiguousarray((HSCALE * DT * J).T).astype(np.float16)
    bmt = np.ascontiguousarray(
        (HSCALE * DT * Bmat[:, 0]).reshape(CB, 128).T
    ).astype(np.float32)
    wrt = np.ascontiguousarray(W_ro[0].reshape(CB, 128).T).astype(np.float16)
    return [
        {
            "JT": jt,
            "bmt": bmt,
            "wrt": wrt,
            "velT": np.ascontiguousarray(
                vel[c * B_LOC : (c + 1) * B_LOC].T
            ).astype(np.float16),
        }
        for c in range(N_CORES)
    ]


def kernel(vel, J, Bmat, W_ro, _trace=False, **build_kw):
    from concourse.bass_utils import run_bass_kernel_spmd

    nc = _get_nc(**build_kw)
    in_maps = make_in_maps(vel, J, Bmat, W_ro)
    res = run_bass_kernel_spmd(nc, in_maps, list(range(N_CORES)), trace=_trace)
    # out[0, t*B_LOC + b] = readout(batch row b, step t)
    out = np.stack(
        [r["out"].reshape(T_FULL, B_LOC).T for r in res.results], axis=0
    ).reshape(BATCH, T_FULL)
    out = out[:, :, None].astype(np.float32)
    if _trace:
        kernel.last_results = res
    return out


kernel.last_results = None


# revision 11
# speedup vs baseline: 1.0261x; 1.0006x over previous
"""CTRNN forward kernel for Trainium2 (8 NeuronCores, data-parallel over batch).

Reference computation (per step t, dt=0.02):
    h = h*(1-dt) + dt*(tanh(h) @ J.T + v_t @ Bmat.T)
    out_t = tanh(h) @ W_ro.T

Design (v3):
  - Per core: B_LOC=16 batch rows, hT layout (hidden on partitions, 4 row
    blocks of 128; batch on free dim). h lives in PSUM scaled by HSCALE=64
    (fp16 subnormal guard): H = 64h, y = tanh(H/64) via ACT's input scale.
  - h is double-buffered across two PSUM bank SETS (X at even steps, Y at
    odd steps), each set = 2 banks of [128, 32] (= 2 row blocks x 16 batch).
    Per step, per half g: one DVE scalar_tensor_tensor
        S_t[g] = 0.98*S_{t-1}[g] + bv_t[g]
    reads the PREVIOUS set, so it depends only on step t-1's matmuls --
    not on ACT -- and ACT(t-1) can read S_{t-1} concurrently.
  - 16 fp16 matmuls/step (J tiles [128,128] stationary, y [128,16] moving)
    accumulate into S_t, then 2 ACTs produce y_t = tanh(S_t/64) into a
    32-step fp16 ring. The MM issue order (PI below) was chosen by
    simulating the steady-state pipeline (MM drain 174ns + sem ~156ns +
    ACT ~273ns + sem ~45ns on the y loop); add_dep_helper(sync=False)
    chains pin the per-engine order against scheduler reordering.
  - bv outer products: vel is broadcast-DMA'd to all 128 partitions once
    per 128-step block, then 16 small DVE tensor_scalar multiplies
    (scalar = per-partition 64*dt*B column) build bvp -- no PE, no PSUM.
  - Readout batched per 32 steps: 4 accumulating MMs (lhsT = W_ro chunk
    [128,1], rhs = y ring [128,512]) -> PSUM [1,512] -> SBUF -> DRAM.
"""

import math
import sys

import numpy as np

sys.path.insert(0, "/opt/trn_rl_repo")

DT = 0.02
DECAY = 1.0 - DT          # 0.98
HSCALE = 64.0             # h kept as 64*h in PSUM (fp16 subnormal guard)
HIDDEN = 512
BATCH = 128
T_FULL = 1024
N_CORES = 8
B_LOC = BATCH // N_CORES  # 16
CB = HIDDEN // 128        # 4 row blocks / y chunks

# MM issue order (block b, chunk c); groups: bank A = blocks {0,1},
# bank B = blocks {2,3}. Found by steady-state pipeline search.
PI = [
    (3, 2), (3, 3), (2, 3), (2, 2), (1, 2), (2, 0), (2, 1), (3, 1),
    (3, 0), (1, 1), (1, 3), (1, 0), (0, 0), (0, 2), (0, 3), (0, 1),
]


def build_nc(T=T_FULL, lbv=128, ro=32):
    import concourse.bass as bass
    import concourse.tile as tile
    from concourse import bacc, mybir

    f32 = mybir.dt.float32
    f16 = mybir.dt.float16
    nc = bacc.Bacc()

    jt_h = nc.declare_dram_parameter("JT", [HIDDEN, HIDDEN], f16, isOutput=False)
    bmt_h = nc.declare_dram_parameter("bmt", [128, CB], f32, isOutput=False)
    wrt_h = nc.declare_dram_parameter("wrt", [128, CB], f16, isOutput=False)
    velt_h = nc.declare_dram_parameter("velT", [T, B_LOC], f16, isOutput=False)
    out_h = nc.declare_dram_parameter("out", [1, T * B_LOC], f32, isOutput=True)

    nblk = (T + lbv - 1) // lbv
    nro = (T + ro - 1) // ro
    rosz = ro * B_LOC  # 512 = one PSUM bank of fp32

    last = {}

    def chain(key, inst):
        last[key] = inst
        return inst

    with tile.TileContext(nc) as tc:
        with (
            tc.tile_pool(name="singles", bufs=1) as singles,
            tc.tile_pool(name="ybp", bufs=2) as ybp,
            tc.tile_pool(name="velp", bufs=2) as velp,
            tc.tile_pool(name="bvpp", bufs=2) as bvpp,
            tc.tile_pool(name="osbp", bufs=2) as osbp,
            tc.tile_pool(name="psum", bufs=1, space="PSUM") as pp,
        ):
            # ---- weights staging ----
            jt = singles.tile([128, CB, HIDDEN], f16, tag="jt")  # (64*dt*J)^T
            nc.sync.dma_start(out=jt, in_=jt_h.rearrange("(c p) i -> p c i", p=128))
            bmt = singles.tile([128, CB], f32, tag="bmt")  # 64*dt*Bmat columns
            nc.sync.dma_start(out=bmt, in_=bmt_h[:, :])
            wrt = singles.tile([128, CB], f16, tag="wrt")
            nc.sync.dma_start(out=wrt, in_=wrt_h[:, :])

            zrow = singles.tile([1, 512], f32, tag="zrow")
            chain("dve", nc.vector.memset(zrow, 0.0))

            y0 = singles.tile([128, CB, B_LOC], f16, tag="y0")
            chain(
                "dve",
                nc.vector.memset(y0.rearrange("p c b -> p (c b)").bitcast(f32), 0.0),
            )

            # h bank sets (3-way rotation): xy[s][g] = [128, 32]
            xy = [
                [
                    pp.tile([128, 2 * B_LOC], f32, tag=f"z{s}{g}", name=f"psum_z{s}{g}")
                    for g in range(2)
                ]
                for s in range(3)
            ]
            pjunk = pp.tile([1, 8], f32, tag="junk", name="psum_junk")

            def absorb(src):
                if src.dtype != f32:
                    src = src.bitcast(f32)
                chain(
                    "pe",
                    nc.tensor.matmul(
                        out=pjunk[0:1, 0:1],
                        lhsT=src,
                        rhs=src,
                        start=True,
                        stop=True,
                        skip_group_check=True,
                    ),
                )

            # claim + zero all six h banks
            for s in range(3):
                for g in range(2):
                    chain(
                        "pe",
                        nc.tensor.matmul(
                            out=xy[s][g],
                            lhsT=zrow[0:1, 0:128],
                            rhs=zrow[0:1, 0 : 2 * B_LOC],
                            start=True,
                            stop=True,
                            skip_group_check=True,
                        ),
                    )

            absorb(jt[0:1, 0, 0:2])
            absorb(wrt[0:1, 0:2])
            absorb(bmt[0:1, 0:1])

            def dma_velb(t0):
                # vel for one lbv block, broadcast to all 128 partitions
                vb = velp.tile([128, lbv * B_LOC], f16, tag="velB")
                nc.sync.dma_start(
                    out=vb,
                    in_=velt_h[t0 : t0 + lbv, :]
                    .rearrange("t b -> (t b)")
                    .unsqueeze(0)
                    .partition_broadcast(128),
                )
                return vb

            def build_bv(r, vb, bvp_t):
                # pair r: chunk c = r // 4, quarter q = r % 4 (32 steps)
                c, q = divmod(r, 4)
                chain(
                    "dve",
                    nc.vector.tensor_scalar_mul(
                        out=bvp_t[:, q * 32 : (q + 1) * 32, c, :],
                        in0=vb[:, q * 512 : (q + 1) * 512].rearrange(
                            "p (t b) -> p t b", b=B_LOC
                        ),
                        scalar1=bmt[:, c : c + 1],
                    ),
                )

            def emit_readout(k, ytile):
                pro = pp.tile([1, rosz], f32, tag="ro", bufs=1, name="psum_ro")
                for c in range(CB):
                    chain(
                        "pe",
                        nc.tensor.matmul(
                            out=pro,
                            lhsT=wrt[:, c : c + 1],
                            rhs=ytile[:, c, :, :].rearrange("p t b -> p (t b)"),
                            start=(c == 0),
                            stop=(c == CB - 1),
                            skip_group_check=True,
                        ),
                    )
                osb = osbp.tile([1, rosz], f32, tag="osb", name="out_sb")
                chain("dve", nc.vector.tensor_copy(osb, pro))
                nc.sync.dma_start(
                    out=out_h[0:1, k * rosz : (k + 1) * rosz], in_=osb
                )

            # prologue: vel block 0 + its bv products
            velb = dma_velb(0)
            bvp_cur = bvpp.tile([128, lbv, CB, B_LOC], f16, tag="bvp")
            for r in range(16):
                build_bv(r, velb, bvp_cur)
            bvp_next = None
            velb_next = None

            yb_cur = None
            yb_prev = None
            for t in range(T):
                blk, j = divmod(t, lbv)
                rob, rj = divmod(t, ro)

                if rj == 0:
                    yb_prev = yb_cur
                    yb_cur = ybp.tile([128, CB, ro, B_LOC], f16, tag="yb")

                if j == 0 and blk + 1 < nblk:
                    velb_next = dma_velb(t + lbv)
                    bvp_next = bvpp.tile([128, lbv, CB, B_LOC], f16, tag="bvp")

                # spread next block's bv build: one op per 8 steps
                if blk + 1 < nblk and j >= 8 and j % 8 == 0:
                    build_bv(j // 8 - 1, velb_next, bvp_next)
                    if j == 120:
                        build_bv(15, velb_next, bvp_next)

                # batched readout of the previous 32-step block
                if rj == 4 and rob >= 1:
                    emit_readout(rob - 1, yb_prev)

                # ---- the step ----
                S = xy[t % 3]
                Pv = xy[(t + 2) % 3]
                # decay+input per quarter-column so each range's last writer
                # is a single full-cover matmul (keeps ACT waits single)
                for b in (2, 3, 1, 0):
                    cs = slice(16 * (b % 2), 16 * (b % 2) + 16)
                    chain(
                        "dve",
                        nc.vector.scalar_tensor_tensor(
                            out=S[b // 2][:, cs],
                            in0=Pv[b // 2][:, cs],
                            scalar=float(DECAY),
                            in1=bvp_cur[:, j, b, :],
                            op0=mybir.AluOpType.mult,
                            op1=mybir.AluOpType.add,
                        ),
                    )
                if t == 0:
                    ysl = lambda c: y0[:, c, :]
                elif rj == 0:
                    ysl = lambda c: yb_prev[:, c, ro - 1, :]
                else:
                    ysl = lambda c, _s=rj - 1: yb_cur[:, c, _s, :]
                for b, c in PI:
                    chain(
                        "pe",
                        nc.tensor.matmul(
                            out=S[b // 2][:, 16 * (b % 2) : 16 * (b % 2) + 16],
                            lhsT=jt[:, c, 128 * b : 128 * (b + 1)],
                            rhs=ysl(c),
                            start=False,
                            stop=False,
                            skip_group_check=True,
                        ),
                    )
                for g in (1, 0):
                    chain(
                        "sce",
                        nc.scalar.activation(
                            out=yb_cur[:, 2 * g : 2 * g + 2, rj, :],
                            in_=S[g].rearrange("p (c b) -> p c b", b=B_LOC),
                            func=mybir.ActivationFunctionType.Tanh,
                            scale=1.0 / HSCALE,
                        ),
                    )

                if j == lbv - 1 and blk + 1 < nblk:
                    bvp_cur = bvp_next
                    velb = velb_next

            emit_readout(nro - 1, yb_cur)

    nc.compile()
    return nc


_NC_CACHE = {}


def _get_nc(**kw):
    key = tuple(sorted(kw.items()))
    if key not in _NC_CACHE:
        _NC_CACHE[key] = build_nc(**kw)
    return _NC_CACHE[key]


def make_in_maps(vel, J, Bmat, W_ro):
    vel = np.asarray(vel, dtype=np.float32)[:, :, 0]          # [B, T]
    J = np.asarray(J, dtype=np.float32)
    Bmat = np.asarray(Bmat, dtype=np.float32)
    W_ro = np.asarray(W_ro, dtype=np.float32)

    jt = np.ascontiguousarray((HSCALE * DT * J).T).astype(np.float16)
    bmt = np.ascontiguousarray(
        (HSCALE * DT * Bmat[:, 0]).reshape(CB, 128).T
    ).astype(np.float32)
    wrt = np.ascontiguousarray(W_ro[0].reshape(CB, 128).T).astype(np.float16)
    return [
        {
            "JT": jt,
            "bmt": bmt,
            "wrt": wrt,
            "velT": np.ascontiguousarray(
                vel[c * B_LOC : (c + 1) * B_LOC].T
            ).astype(np.float16),
        }
        for c in range(N_CORES)
    ]


def kernel(vel, J, Bmat, W_ro, _trace=False, **build_kw):
    from concourse.bass_utils import run_bass_kernel_spmd

    nc = _get_nc(**build_kw)
    in_maps = make_in_maps(vel, J, Bmat, W_ro)
    res = run_bass_kernel_spmd(nc, in_maps, list(range(N_CORES)), trace=_trace)
    # out[0, t*B_LOC + b] = readout(batch row b, step t)
    out = np.stack(
        [r["out"].reshape(T_FULL, B_LOC).T for r in res.results], axis=0
    ).reshape(BATCH, T_FULL)
    out = out[:, :, None].astype(np.float32)
    if _trace:
        kernel.last_results = res
    return out


kernel.last_results = None


# revision 12
# speedup vs baseline: 1.2328x; 1.2015x over previous
"""CTRNN forward kernel for Trainium2 (8 NeuronCores, data-parallel over batch).

Reference computation (per step t, dt=0.02):
    h = h*(1-dt) + dt*(tanh(h) @ J.T + v_t @ Bmat.T)
    out_t = tanh(h) @ W_ro.T

Design (v3):
  - Per core: B_LOC=16 batch rows, hT layout (hidden on partitions, 4 row
    blocks of 128; batch on free dim). h lives in PSUM scaled by HSCALE=64
    (fp16 subnormal guard): H = 64h, y = tanh(H/64) via ACT's input scale.
  - h is double-buffered across two PSUM bank SETS (X at even steps, Y at
    odd steps), each set = 2 banks of [128, 32] (= 2 row blocks x 16 batch).
    Per step, per half g: one DVE scalar_tensor_tensor
        S_t[g] = 0.98*S_{t-1}[g] + bv_t[g]
    reads the PREVIOUS set, so it depends only on step t-1's matmuls --
    not on ACT -- and ACT(t-1) can read S_{t-1} concurrently.
  - 16 fp16 matmuls/step (J tiles [128,128] stationary, y [128,16] moving)
    accumulate into S_t, then 2 ACTs produce y_t = tanh(S_t/64) into a
    32-step fp16 ring. The MM issue order (PI below) was chosen by
    simulating the steady-state pipeline (MM drain 174ns + sem ~156ns +
    ACT ~273ns + sem ~45ns on the y loop); emission order sets scheduler
    priorities (explicit dep chains measured slower -- extra sem traffic).
  - bv outer products: vel is broadcast-DMA'd to all 128 partitions once
    per 128-step block, then 16 small DVE tensor_scalar multiplies
    (scalar = per-partition 64*dt*B column) build bvp -- no PE, no PSUM.
  - Readout batched per 32 steps: 4 accumulating MMs (lhsT = W_ro chunk
    [128,1], rhs = y ring [128,512]) -> PSUM [1,512] -> SBUF -> DRAM.
"""

import math
import sys

import numpy as np

sys.path.insert(0, "/opt/trn_rl_repo")

DT = 0.02
DECAY = 1.0 - DT          # 0.98
HSCALE = 64.0             # h kept as 64*h in PSUM (fp16 subnormal guard)
HIDDEN = 512
BATCH = 128
T_FULL = 1024
N_CORES = 8
B_LOC = BATCH // N_CORES  # 16
CB = HIDDEN // 128        # 4 row blocks / y chunks

# MM issue order (block b, chunk c); groups: bank A = blocks {0,1},
# bank B = blocks {2,3}. Found by steady-state pipeline search.
PI = [
    (1, 1), (0, 1), (1, 0), (2, 0), (2, 1), (0, 0), (0, 3), (1, 2),
    (0, 2), (1, 3), (3, 0), (3, 2), (3, 1), (3, 3), (2, 3), (2, 2),
]


def build_nc(T=T_FULL, lbv=128, ro=32):
    import concourse.bass as bass
    import concourse.tile as tile
    from concourse import bacc, mybir

    f32 = mybir.dt.float32
    f16 = mybir.dt.float16
    nc = bacc.Bacc()

    jt_h = nc.declare_dram_parameter("JT", [HIDDEN, HIDDEN], f16, isOutput=False)
    bmt_h = nc.declare_dram_parameter("bmt", [128, CB], f32, isOutput=False)
    wrt_h = nc.declare_dram_parameter("wrt", [128, CB], f16, isOutput=False)
    velt_h = nc.declare_dram_parameter("velT", [T, B_LOC], f16, isOutput=False)
    out_h = nc.declare_dram_parameter("out", [1, T * B_LOC], f32, isOutput=True)

    nblk = (T + lbv - 1) // lbv
    nro = (T + ro - 1) // ro
    rosz = ro * B_LOC  # 512 = one PSUM bank of fp32

    last = {}

    def chain(key, inst):
        last[key] = inst
        return inst

    with tile.TileContext(nc) as tc:
        with (
            tc.tile_pool(name="singles", bufs=1) as singles,
            tc.tile_pool(name="ybp", bufs=2) as ybp,
            tc.tile_pool(name="velp", bufs=2) as velp,
            tc.tile_pool(name="bvpp", bufs=2) as bvpp,
            tc.tile_pool(name="osbp", bufs=2) as osbp,
            tc.tile_pool(name="psum", bufs=1, space="PSUM") as pp,
        ):
            # ---- weights staging ----
            jt = singles.tile([128, CB, HIDDEN], f16, tag="jt")  # (64*dt*J)^T
            nc.sync.dma_start(out=jt, in_=jt_h.rearrange("(c p) i -> p c i", p=128))
            bmt = singles.tile([128, CB], f32, tag="bmt")  # 64*dt*Bmat columns
            nc.sync.dma_start(out=bmt, in_=bmt_h[:, :])
            wrt = singles.tile([128, CB], f16, tag="wrt")
            nc.sync.dma_start(out=wrt, in_=wrt_h[:, :])

            zrow = singles.tile([1, 512], f32, tag="zrow")
            chain("dve", nc.vector.memset(zrow, 0.0))

            y0 = singles.tile([128, CB, B_LOC], f16, tag="y0")
            chain(
                "dve",
                nc.vector.memset(y0.rearrange("p c b -> p (c b)").bitcast(f32), 0.0),
            )

            # h bank sets: xy[s][g] = [128, 32] (blocks 2g, 2g+1)
            xy = [
                [
                    pp.tile([128, 2 * B_LOC], f32, tag=f"z{s}{g}", name=f"psum_z{s}{g}")
                    for g in range(2)
                ]
                for s in range(2)
            ]
            pjunk = pp.tile([1, 8], f32, tag="junk", name="psum_junk")

            def absorb(src):
                if src.dtype != f32:
                    src = src.bitcast(f32)
                chain(
                    "pe",
                    nc.tensor.matmul(
                        out=pjunk[0:1, 0:1],
                        lhsT=src,
                        rhs=src,
                        start=True,
                        stop=True,
                        skip_group_check=True,
                    ),
                )

            # claim + zero all four h banks
            for s in range(2):
                for g in range(2):
                    chain(
                        "pe",
                        nc.tensor.matmul(
                            out=xy[s][g],
                            lhsT=zrow[0:1, 0:128],
                            rhs=zrow[0:1, 0 : 2 * B_LOC],
                            start=True,
                            stop=True,
                            skip_group_check=True,
                        ),
                    )

            absorb(jt[0:1, 0, 0:2])
            absorb(wrt[0:1, 0:2])
            absorb(bmt[0:1, 0:1])

            def dma_velb(t0):
                # vel for one lbv block, broadcast to all 128 partitions
                vb = velp.tile([128, lbv * B_LOC], f16, tag="velB")
                nc.sync.dma_start(
                    out=vb,
                    in_=velt_h[t0 : t0 + lbv, :]
                    .rearrange("t b -> (t b)")
                    .unsqueeze(0)
                    .partition_broadcast(128),
                )
                return vb

            def build_bv(r, vb, bvp_t):
                # pair r: chunk c = r // 4, quarter q = r % 4 (32 steps)
                c, q = divmod(r, 4)
                chain(
                    "dve",
                    nc.vector.tensor_scalar_mul(
                        out=bvp_t[:, q * 32 : (q + 1) * 32, c, :],
                        in0=vb[:, q * 512 : (q + 1) * 512].rearrange(
                            "p (t b) -> p t b", b=B_LOC
                        ),
                        scalar1=bmt[:, c : c + 1],
                    ),
                )

            def emit_readout(k, ytile):
                pro = pp.tile([1, rosz], f32, tag="ro", bufs=1, name="psum_ro")
                for c in range(CB):
                    chain(
                        "pe",
                        nc.tensor.matmul(
                            out=pro,
                            lhsT=wrt[:, c : c + 1],
                            rhs=ytile[:, c, :, :].rearrange("p t b -> p (t b)"),
                            start=(c == 0),
                            stop=(c == CB - 1),
                            skip_group_check=True,
                        ),
                    )
                osb = osbp.tile([1, rosz], f32, tag="osb", name="out_sb")
                chain("dve", nc.vector.tensor_copy(osb, pro))
                nc.sync.dma_start(
                    out=out_h[0:1, k * rosz : (k + 1) * rosz], in_=osb
                )

            # prologue: vel block 0 + its bv products
            velb = dma_velb(0)
            bvp_cur = bvpp.tile([128, lbv, CB, B_LOC], f16, tag="bvp")
            for r in range(16):
                build_bv(r, velb, bvp_cur)
            bvp_next = None
            velb_next = None

            yb_cur = None
            yb_prev = None
            for t in range(T):
                blk, j = divmod(t, lbv)
                rob, rj = divmod(t, ro)

                if rj == 0:
                    yb_prev = yb_cur
                    yb_cur = ybp.tile([128, CB, ro, B_LOC], f16, tag="yb")

                if j == 0 and blk + 1 < nblk:
                    velb_next = dma_velb(t + lbv)
                    bvp_next = bvpp.tile([128, lbv, CB, B_LOC], f16, tag="bvp")

                # spread next block's bv build: one op per 8 steps
                if blk + 1 < nblk and j >= 8 and j % 8 == 0:
                    build_bv(j // 8 - 1, velb_next, bvp_next)
                    if j == 120:
                        build_bv(15, velb_next, bvp_next)

                # batched readout of the previous 32-step block
                if rj == 4 and rob >= 1:
                    emit_readout(rob - 1, yb_prev)

                # ---- the step ----
                S = xy[t % 2]
                Pv = xy[1 - t % 2]
                for g in range(2):
                    chain(
                        "dve",
                        nc.vector.scalar_tensor_tensor(
                            out=S[g],
                            in0=Pv[g],
                            scalar=float(DECAY),
                            in1=bvp_cur[:, j, 2 * g : 2 * g + 2, :].rearrange(
                                "p c b -> p (c b)"
                            ),
                            op0=mybir.AluOpType.mult,
                            op1=mybir.AluOpType.add,
                        ),
                    )
                if t == 0:
                    ysl = lambda c: y0[:, c, :]
                elif rj == 0:
                    ysl = lambda c: yb_prev[:, c, ro - 1, :]
                else:
                    ysl = lambda c, _s=rj - 1: yb_cur[:, c, _s, :]
                for b, c in PI:
                    chain(
                        "pe",
                        nc.tensor.matmul(
                            out=S[b // 2][:, 16 * (b % 2) : 16 * (b % 2) + 16],
                            lhsT=jt[:, c, 128 * b : 128 * (b + 1)],
                            rhs=ysl(c),
                            start=False,
                            stop=False,
                            skip_group_check=True,
                        ),
                    )
                for g in range(2):
                    chain(
                        "sce",
                        nc.scalar.activation(
                            out=yb_cur[:, 2 * g : 2 * g + 2, rj, :],
                            in_=S[g].rearrange("p (c b) -> p c b", b=B_LOC),
                            func=mybir.ActivationFunctionType.Tanh,
                            scale=1.0 / HSCALE,
                        ),
                    )

                if j == lbv - 1 and blk + 1 < nblk:
                    bvp_cur = bvp_next
                    velb = velb_next

            emit_readout(nro - 1, yb_cur)

    nc.compile()
    return nc


_NC_CACHE = {}


def _get_nc(**kw):
    key = tuple(sorted(kw.items()))
    if key not in _NC_CACHE:
        _NC_CACHE[key] = build_nc(**kw)
    return _NC_CACHE[key]


def make_in_maps(vel, J, Bmat, W_ro):
    vel = np.asarray(vel, dtype=np.float32)[:, :, 0]          # [B, T]
    J = np.asarray(J, dtype=np.float32)
    Bmat = np.asarray(Bmat, dtype=np.float32)
    W_ro = np.asarray(W_ro, dtype=np.float32)

    jt = np.ascontiguousarray((HSCALE * DT * J).T).astype(np.float16)
    bmt = np.ascontiguousarray(
        (HSCALE * DT * Bmat[:, 0]).reshape(CB, 128).T
    ).astype(np.float32)
    wrt = np.ascontiguousarray(W_ro[0].reshape(CB, 128).T).astype(np.float16)
    return [
        {
            "JT": jt,
            "bmt": bmt,
            "wrt": wrt,
            "velT": np.ascontiguousarray(
                vel[c * B_LOC : (c + 1) * B_LOC].T
            ).astype(np.float16),
        }
        for c in range(N_CORES)
    ]


def kernel(vel, J, Bmat, W_ro, _trace=False, **build_kw):
    from concourse.bass_utils import run_bass_kernel_spmd

    nc = _get_nc(**build_kw)
    in_maps = make_in_maps(vel, J, Bmat, W_ro)
    res = run_bass_kernel_spmd(nc, in_maps, list(range(N_CORES)), trace=_trace)
    # out[0, t*B_LOC + b] = readout(batch row b, step t)
    out = np.stack(
        [r["out"].reshape(T_FULL, B_LOC).T for r in res.results], axis=0
    ).reshape(BATCH, T_FULL)
    out = out[:, :, None].astype(np.float32)
    if _trace:
        kernel.last_results = res
    return out


kernel.last_results = None


# revision 13
# speedup vs baseline: 1.2337x; 1.0007x over previous
"""CTRNN forward kernel for Trainium2 (8 NeuronCores, data-parallel over batch).

Reference computation (per step t, dt=0.02):
    h = h*(1-dt) + dt*(tanh(h) @ J.T + v_t @ Bmat.T)
    out_t = tanh(h) @ W_ro.T

Design (v3):
  - Per core: B_LOC=16 batch rows, hT layout (hidden on partitions, 4 row
    blocks of 128; batch on free dim). h lives in PSUM scaled by HSCALE=64
    (fp16 subnormal guard): H = 64h, y = tanh(H/64) via ACT's input scale.
  - h is double-buffered across two PSUM bank SETS (X at even steps, Y at
    odd steps), each set = 2 banks of [128, 32] (= 2 row blocks x 16 batch).
    Per step, per half g: one DVE scalar_tensor_tensor
        S_t[g] = 0.98*S_{t-1}[g] + bv_t[g]
    reads the PREVIOUS set, so it depends only on step t-1's matmuls --
    not on ACT -- and ACT(t-1) can read S_{t-1} concurrently.
  - 16 fp16 matmuls/step (J tiles [128,128] stationary, y [128,16] moving)
    accumulate into S_t, then 2 ACTs produce y_t = tanh(S_t/64) into a
    32-step fp16 ring. The MM issue order (PI below) was chosen by
    simulating the steady-state pipeline (MM drain 174ns + sem ~156ns +
    ACT ~273ns + sem ~45ns on the y loop); emission order sets scheduler
    priorities (explicit dep chains measured slower -- extra sem traffic).
  - bv outer products: vel is broadcast-DMA'd to all 128 partitions once
    per 128-step block, then 16 small DVE tensor_scalar multiplies
    (scalar = per-partition 64*dt*B column) build bvp -- no PE, no PSUM.
  - Readout batched per 32 steps: 4 accumulating MMs (lhsT = W_ro chunk
    [128,1], rhs = y ring [128,512]) -> PSUM [1,512] -> SBUF -> DRAM.
"""

import math
import sys

import numpy as np

sys.path.insert(0, "/opt/trn_rl_repo")

DT = 0.02
DECAY = 1.0 - DT          # 0.98
HSCALE = 64.0             # h kept as 64*h in PSUM (fp16 subnormal guard)
HIDDEN = 512
BATCH = 128
T_FULL = 1024
N_CORES = 8
B_LOC = BATCH // N_CORES  # 16
CB = HIDDEN // 128        # 4 row blocks / y chunks

# MM issue order (block b, chunk c); groups: bank A = blocks {0,1},
# bank B = blocks {2,3}. Found by steady-state pipeline search.
PI = [
    (1, 1), (0, 1), (1, 0), (2, 0), (2, 1), (0, 0), (0, 3), (1, 2),
    (0, 2), (1, 3), (3, 0), (3, 2), (3, 1), (3, 3), (2, 3), (2, 2),
]


def build_nc(T=T_FULL, lbv=128, ro=32):
    import concourse.bass as bass
    import concourse.tile as tile
    from concourse import bacc, mybir

    f32 = mybir.dt.float32
    f16 = mybir.dt.float16
    nc = bacc.Bacc()

    jt_h = nc.declare_dram_parameter("JT", [HIDDEN, HIDDEN], f16, isOutput=False)
    bmt_h = nc.declare_dram_parameter("bmt", [128, CB], f32, isOutput=False)
    wrt_h = nc.declare_dram_parameter("wrt", [128, CB], f16, isOutput=False)
    velt_h = nc.declare_dram_parameter("velT", [T, B_LOC], f16, isOutput=False)
    out_h = nc.declare_dram_parameter("out", [1, T * B_LOC], f32, isOutput=True)

    nblk = (T + lbv - 1) // lbv
    nro = (T + ro - 1) // ro
    rosz = ro * B_LOC  # 512 = one PSUM bank of fp32

    last = {}

    def chain(key, inst):
        last[key] = inst
        return inst

    with tile.TileContext(nc) as tc:
        with (
            tc.tile_pool(name="singles", bufs=1) as singles,
            tc.tile_pool(name="ybp", bufs=2) as ybp,
            tc.tile_pool(name="velp", bufs=2) as velp,
            tc.tile_pool(name="bvpp", bufs=2) as bvpp,
            tc.tile_pool(name="osbp", bufs=2) as osbp,
            tc.tile_pool(name="psum", bufs=1, space="PSUM") as pp,
        ):
            # ---- weights staging ----
            jt = singles.tile([128, CB, HIDDEN], f16, tag="jt")  # (64*dt*J)^T
            nc.sync.dma_start(out=jt, in_=jt_h.rearrange("(c p) i -> p c i", p=128))
            bmt = singles.tile([128, CB], f32, tag="bmt")  # 64*dt*Bmat columns
            nc.sync.dma_start(out=bmt, in_=bmt_h[:, :])
            wrt = singles.tile([128, CB], f16, tag="wrt")
            nc.sync.dma_start(out=wrt, in_=wrt_h[:, :])

            zrow = singles.tile([1, 512], f32, tag="zrow")
            chain("dve", nc.vector.memset(zrow, 0.0))

            y0 = singles.tile([128, CB, B_LOC], f16, tag="y0")
            chain(
                "dve",
                nc.vector.memset(y0.rearrange("p c b -> p (c b)").bitcast(f32), 0.0),
            )

            # h bank sets: xy[s][g] = [128, 32] (blocks 2g, 2g+1)
            xy = [
                [
                    pp.tile([128, 2 * B_LOC], f32, tag=f"z{s}{g}", name=f"psum_z{s}{g}")
                    for g in range(2)
                ]
                for s in range(3)
            ]
            pjunk = pp.tile([1, 8], f32, tag="junk", name="psum_junk")

            def absorb(src):
                if src.dtype != f32:
                    src = src.bitcast(f32)
                chain(
                    "pe",
                    nc.tensor.matmul(
                        out=pjunk[0:1, 0:1],
                        lhsT=src,
                        rhs=src,
                        start=True,
                        stop=True,
                        skip_group_check=True,
                    ),
                )

            # claim + zero all four h banks
            for s in range(3):
                for g in range(2):
                    chain(
                        "pe",
                        nc.tensor.matmul(
                            out=xy[s][g],
                            lhsT=zrow[0:1, 0:128],
                            rhs=zrow[0:1, 0 : 2 * B_LOC],
                            start=True,
                            stop=True,
                            skip_group_check=True,
                        ),
                    )

            absorb(jt[0:1, 0, 0:2])
            absorb(wrt[0:1, 0:2])
            absorb(bmt[0:1, 0:1])

            def dma_velb(t0):
                # vel for one lbv block, broadcast to all 128 partitions
                vb = velp.tile([128, lbv * B_LOC], f16, tag="velB")
                nc.sync.dma_start(
                    out=vb,
                    in_=velt_h[t0 : t0 + lbv, :]
                    .rearrange("t b -> (t b)")
                    .unsqueeze(0)
                    .partition_broadcast(128),
                )
                return vb

            def build_bv(r, vb, bvp_t):
                # pair r: chunk c = r // 4, quarter q = r % 4 (32 steps)
                c, q = divmod(r, 4)
                chain(
                    "dve",
                    nc.vector.tensor_scalar_mul(
                        out=bvp_t[:, q * 32 : (q + 1) * 32, c, :],
                        in0=vb[:, q * 512 : (q + 1) * 512].rearrange(
                            "p (t b) -> p t b", b=B_LOC
                        ),
                        scalar1=bmt[:, c : c + 1],
                    ),
                )

            def emit_readout(k, ytile):
                pro = pp.tile([1, rosz], f32, tag="ro", bufs=1, name="psum_ro")
                for c in range(CB):
                    chain(
                        "pe",
                        nc.tensor.matmul(
                            out=pro,
                            lhsT=wrt[:, c : c + 1],
                            rhs=ytile[:, c, :, :].rearrange("p t b -> p (t b)"),
                            start=(c == 0),
                            stop=(c == CB - 1),
                            skip_group_check=True,
                        ),
                    )
                osb = osbp.tile([1, rosz], f32, tag="osb", name="out_sb")
                chain("dve", nc.vector.tensor_copy(osb, pro))
                nc.sync.dma_start(
                    out=out_h[0:1, k * rosz : (k + 1) * rosz], in_=osb
                )

            # prologue: vel block 0 + its bv products
            velb = dma_velb(0)
            bvp_cur = bvpp.tile([128, lbv, CB, B_LOC], f16, tag="bvp")
            for r in range(16):
                build_bv(r, velb, bvp_cur)
            bvp_next = None
            velb_next = None

            yb_cur = None
            yb_prev = None
            for t in range(T):
                blk, j = divmod(t, lbv)
                rob, rj = divmod(t, ro)

                if rj == 0:
                    yb_prev = yb_cur
                    yb_cur = ybp.tile([128, CB, ro, B_LOC], f16, tag="yb")

                if j == 0 and blk + 1 < nblk:
                    velb_next = dma_velb(t + lbv)
                    bvp_next = bvpp.tile([128, lbv, CB, B_LOC], f16, tag="bvp")

                # spread next block's bv build: one op per 8 steps
                if blk + 1 < nblk and j >= 8 and j % 8 == 0:
                    build_bv(j // 8 - 1, velb_next, bvp_next)
                    if j == 120:
                        build_bv(15, velb_next, bvp_next)

                # batched readout of the previous 32-step block
                if rj == 4 and rob >= 1:
                    emit_readout(rob - 1, yb_prev)

                # ---- the step ----
                S = xy[t % 3]
                Pv = xy[(t + 2) % 3]
                for g in range(2):
                    chain(
                        "dve",
                        nc.vector.scalar_tensor_tensor(
                            out=S[g],
                            in0=Pv[g],
                            scalar=float(DECAY),
                            in1=bvp_cur[:, j, 2 * g : 2 * g + 2, :].rearrange(
                                "p c b -> p (c b)"
                            ),
                            op0=mybir.AluOpType.mult,
                            op1=mybir.AluOpType.add,
                        ),
                    )
                if t == 0:
                    ysl = lambda c: y0[:, c, :]
                elif rj == 0:
                    ysl = lambda c: yb_prev[:, c, ro - 1, :]
                else:
                    ysl = lambda c, _s=rj - 1: yb_cur[:, c, _s, :]
                for b, c in PI:
                    chain(
                        "pe",
                        nc.tensor.matmul(
                            out=S[b // 2][:, 16 * (b % 2) : 16 * (b % 2) + 16],
                            lhsT=jt[:, c, 128 * b : 128 * (b + 1)],
                            rhs=ysl(c),
                            start=False,
                            stop=False,
                            skip_group_check=True,
                        ),
                    )
                for g in range(2):
                    chain(
                        "sce",
                        nc.scalar.activation(
                            out=yb_cur[:, 2 * g : 2 * g + 2, rj, :],
                            in_=S[g].rearrange("p (c b) -> p c b", b=B_LOC),
                            func=mybir.ActivationFunctionType.Tanh,
                            scale=1.0 / HSCALE,
                        ),
                    )

                if j == lbv - 1 and blk + 1 < nblk:
                    bvp_cur = bvp_next
                    velb = velb_next

            emit_readout(nro - 1, yb_cur)

    nc.compile()
    return nc


_NC_CACHE = {}


def _get_nc(**kw):
    key = tuple(sorted(kw.items()))
    if key not in _NC_CACHE:
        _NC_CACHE[key] = build_nc(**kw)
    return _NC_CACHE[key]


def make_in_maps(vel, J, Bmat, W_ro):
    vel = np.asarray(vel, dtype=np.float32)[:, :, 0]          # [B, T]
    J = np.asarray(J, dtype=np.float32)
    Bmat = np.asarray(Bmat, dtype=np.float32)
    W_ro = np.asarray(W_ro, dtype=np.float32)

    jt = np.ascontiguousarray((HSCALE * DT * J).T).astype(np.float16)
    bmt = np.ascontiguousarray(
        (HSCALE * DT * Bmat[:, 0]).reshape(CB, 128).T
    ).astype(np.float32)
    wrt = np.ascontiguousarray(W_ro[0].reshape(CB, 128).T).astype(np.float16)
    return [
        {
            "JT": jt,
            "bmt": bmt,
            "wrt": wrt,
            "velT": np.ascontiguousarray(
                vel[c * B_LOC : (c + 1) * B_LOC].T
            ).astype(np.float16),
        }
        for c in range(N_CORES)
    ]


def kernel(vel, J, Bmat, W_ro, _trace=False, **build_kw):
    from concourse.bass_utils import run_bass_kernel_spmd

    nc = _get_nc(**build_kw)
    in_maps = make_in_maps(vel, J, Bmat, W_ro)
    res = run_bass_kernel_spmd(nc, in_maps, list(range(N_CORES)), trace=_trace)
    # out[0, t*B_LOC + b] = readout(batch row b, step t)
    out = np.stack(
        [r["out"].reshape(T_FULL, B_LOC).T for r in res.results], axis=0
    ).reshape(BATCH, T_FULL)
    out = out[:, :, None].astype(np.float32)
    if _trace:
        kernel.last_results = res
    return out


kernel.last_results = None
